# revision 43
# baseline (speedup 1.0000x reference)
"""Multi-head cross attention on 8 trn2 NeuronCores.

Problem: B=2, T=4096, EMB=512, H=8 heads (head dim 64), fp32 I/O.
  q = x1 @ Wq.T + bq ; k,v from x2 ; S = q k^T / sqrt(512) ;
  softmax over keys with -1e10 masking ; out = (A v) @ Wu.T + bu.

Sharding: core c handles batch b = c//4 and query rows
[1024*(c%4), 1024*(c%4+1)).  Each core computes K,V for its batch in
full (4-way duplication), its own Q chunk, attention, and out-proj.

Device-side layout choices:
  - All matmul operands fp16 (PE rate is dtype-independent; fp16 halves
    DMA/SBUF and keeps ~1e-3 accuracy), accumulation fp32 in PSUM.
  - Scores computed TRANSPOSED, S^T[key, query]: contraction over the
    head dim requires Q^T/K^T (head-dim on partitions), which fall out
    of computing the projections transposed from x^T inputs (host
    pre-transposes x1/x2/W).  With keys on partitions, P^T = exp(S^T)*M^T
    feeds the AV matmul directly as its stationary-side contraction
    without any on-chip transposes.
  - Scores are small (|S| < ~1) so exp needs no max-subtraction; the
    1/sqrt(512) scale is folded into the ACT exp instruction.
  - V is stored interleaved [key, head, 65] with a ones column so the
    AV matmul also produces the softmax denominators r[q] (row 64).
  - Normalization is deferred: Y^T_h / r_h via reciprocal + a K=1
    broadcast matmul + one DVE multiply per (head, chunk).
  - 2 heads are packed per scores pass via tile_position row-tiling
    (contraction=64 -> rows 0-63 / 64-127 run concurrently).

Schedule (the exp stream on the scalar/ACT engine is the serial floor,
~1.04us per [128,1024] tile, ~265us total -- everything else hides
under it):
  - (key tile, query block) steps with the scores PSUM double-buffered
    (2 banks x 2 bufs) so exp never waits on the next scores matmul;
    AV matmuls issue 3 steps late so the av-buffer WAR at pair starts
    never head-blocks the PE queue.
  - Projections overlap attention: Q + K[e=0] run up front (DMA issue
    order tuned so the first matmul starts ~9us in), V tiles and
    K[e=1..3] are deferred jobs consumed one per odd step (pair 0
    carries V + K[1]; pairs 1/2 carry K[2]/K[3]) riding PE slack under
    the ACT pace; job PSUM shares the scores tag ring.
  - Out-projection alternates two PSUM tags for a 4-deep ring.
"""
import math
import os
from contextlib import ExitStack

import numpy as np

import concourse.bass as bass
import concourse.bacc as bacc
import concourse.tile as tile
import concourse.mybir as mybir
from concourse.bass_utils import run_bass_kernel_spmd

F16 = mybir.dt.float16
F32 = mybir.dt.float32
EXP = mybir.ActivationFunctionType.Exp

EMB, H, D, CT = 512, 8, 64, 4  # emb, heads, head dim, emb/128

FULL_CFG = dict(T=4096, QC=1024)  # keys per batch, query rows per core
MINI_CFG = dict(T=512, QC=256)


def attention_body(ctx, tc, io, cfg):
    nc = tc.nc
    T, QC = cfg["T"], cfg["QC"]
    KT = T // 128            # key tiles
    NG = KT // 2             # key-tile groups of 2
    CH = min(512, QC)        # query chunk width
    NCH = QC // CH
    scale = 1.0 / math.sqrt(EMB)

    pw = ctx.enter_context(tc.tile_pool(name="w", bufs=1))
    pk = ctx.enter_context(tc.tile_pool(name="kt", bufs=1))
    pv = ctx.enter_context(tc.tile_pool(name="v", bufs=1))
    pq = ctx.enter_context(tc.tile_pool(name="qt", bufs=1))

    # persistent weights / biases / constants
    wq = [pw.tile([128, EMB], F16, tag=f"wq{i}", name=f"wq{i}") for i in range(CT)]
    wk = [pw.tile([128, EMB], F16, tag=f"wk{i}", name=f"wk{i}") for i in range(CT)]
    wv = [pw.tile([128, EMB], F16, tag=f"wv{i}", name=f"wv{i}") for i in range(CT)]
    wu = [pw.tile([128, EMB], F16, tag=f"wu{i}", name=f"wu{i}") for i in range(CT)]
    # DMA priority order: what phase A needs first (wq+x1 for Q, then
    # wk+x2 for K e=0); V/out weights follow, mask prefetch queues after.
    bqr = pw.tile([128, CT], F32, tag="bqr", name="bqr")
    bkr = pw.tile([128, CT], F32, tag="bkr", name="bkr")
    bvb = pw.tile([128, EMB], F32, tag="bvb", name="bvb")
    bub = pw.tile([128, EMB], F32, tag="bub", name="bub")
    ones = pw.tile([1, D], F16, tag="ones", name="ones")
    nc.vector.memset(ones[:], 1.0)

    # persistent K^T [emb, T], V [T, head, 65(+pad)], Q^T [emb, QC]
    kt = [pk.tile([128, T], F16, tag=f"kt{i}", name=f"kt{i}") for i in range(CT)]
    v = pv.tile([128, KT, H, 66], F16, tag="v", name="v")
    nc.vector.memset(v[:, :, :, 64:65], 1.0)
    qt = [pq.tile([128, QC], F16, tag=f"qt{i}", name=f"qt{i}") for i in range(CT)]

    px = ctx.enter_context(tc.tile_pool(name="x", bufs=1))
    x2t = [px.tile([128, T], F16, tag=f"x2t{i}", name=f"x2t{i}") for i in range(CT)]
    x1t = [px.tile([128, QC], F16, tag=f"x1t{i}", name=f"x1t{i}") for i in range(CT)]
    # DMA issue order is the critical path to the first matmul: each
    # dma_start costs ~600ns of sync-engine descriptor gen, so x1/wq
    # interleave first (Q proj c=0 starts ~9us in), then wk + x2
    # quarter 1 (K e=0 tb=0), then the rest.
    for i in range(CT):
        nc.sync.dma_start(x1t[i][:], io["x1T"][bass.ts(i, 128), :])
        nc.sync.dma_start(wq[i][:], io["wqT"][bass.ts(i, 128), :])
    nc.sync.dma_start(bqr[:], io["bqr"][:, :])
    for i in range(CT):
        nc.sync.dma_start(wk[i][:], io["wkT"][bass.ts(i, 128), :])
    nc.sync.dma_start(bkr[:], io["bkr"][:, :])
    for hf in range(4):   # quarters: K chunk tb only needs its quarter
        for i in range(CT):
            nc.sync.dma_start(x2t[i][:, bass.ts(hf, T // 4)],
                              io["x2T"][bass.ts(i, 128), bass.ts(hf, T // 4)])

    # attention layout: (key tile, query block) steps.  Scores PSUM is
    # double-buffered at [128, 2*QW] (2 banks x 2 bufs) so the exp ACT
    # stream (the hard serial floor) runs back-to-back.  Projections are
    # folded into the same "s" PSUM tag: Q and K[e=0] run up front, the
    # V tiles interleave into pair 0's steps one tile per step, and
    # K[e=pr+1] runs in the pair boundary where ACT drains anyway.
    QW = min(512, QC)        # matmul moving width (PSUM bank limit)
    NB = QC // QW            # query blocks
    ps_s = ctx.enter_context(tc.tile_pool(name="ps_s", bufs=3, space="PSUM"))
    ps_av = ctx.enter_context(tc.tile_pool(name="ps_av", bufs=2, space="PSUM"))
    pe = ctx.enter_context(tc.tile_pool(name="pe", bufs=3))
    ppp = ctx.enter_context(tc.tile_pool(name="ppp", bufs=6))
    pm = ctx.enter_context(tc.tile_pool(name="pm", bufs=min(KT, 6)))
    py = ctx.enter_context(tc.tile_pool(name="py", bufs=CT))
    pys = ctx.enter_context(tc.tile_pool(name="pys", bufs=2))
    prr = ctx.enter_context(tc.tile_pool(name="prr", bufs=2))
    po = ctx.enter_context(tc.tile_pool(name="po", bufs=3))

    # ---- phase A: Q^T (all e) and K^T e=0 ----
    for e in range(CT):
        ps = ps_s.tile([128, 2 * CH], F32, tag="s", name="psq")
        for c in range(CT):
            for t in range(QC // CH):
                nc.tensor.matmul(ps[:, bass.ts(t, CH)],
                                 wq[c][:, bass.ts(e, 128)],
                                 x1t[c][:, bass.ts(t, CH)],
                                 start=(c == 0), stop=(c == CT - 1))
        nc.vector.tensor_scalar_add(qt[e][:], ps[:, 0:QC], bqr[:, e:e + 1])
    NKC = T // CH // 2       # K chunk pairs per e-tile
    for tb in range(NKC):
        ps = ps_s.tile([128, 2 * CH], F32, tag="s", name="psk")
        for c in range(CT):
            for j in range(2):
                nc.tensor.matmul(ps[:, bass.ts(j, CH)], wk[c][:, 0:128],
                                 x2t[c][:, bass.ts(tb * 2 + j, CH)],
                                 start=(c == 0), stop=(c == CT - 1))
        nc.vector.tensor_scalar_add(kt[0][:, bass.ds(tb * 2 * CH, 2 * CH)],
                                    ps[:], bkr[:, 0:1])

    # remaining weights / biases can load after phase A is under way
    for i in range(CT):
        nc.sync.dma_start(wv[i][:], io["wvT"][bass.ts(i, 128), :])
    nc.sync.dma_start(bvb[:], io["bvb"][:, :])
    for i in range(CT):
        nc.sync.dma_start(wu[i][:], io["wuT"][bass.ts(i, 128), :])
    nc.sync.dma_start(bub[:], io["bub"][:, :])

    # ---- deferred projection jobs (ride odd-step PE slack) ----
    def v_job(t):
        def run():
            ps = ps_s.tile([128, EMB], F32, tag="s", name="psv")
            for c in range(CT):
                nc.tensor.matmul(ps[:], x2t[c][:, bass.ts(t, 128)], wv[c][:],
                                 start=(c == 0), stop=(c == CT - 1))
            nc.vector.tensor_add(
                v[:, t, :, 0:64],
                ps[:].rearrange("p (h d) -> p h d", h=H),
                bvb[:].rearrange("p (h d) -> p h d", h=H))
        return run

    def k_half_job(e, tb, j):
        def run():
            ps = ps_s.tile([128, CH], F32, tag="s", name="psk")
            for c in range(CT):
                nc.tensor.matmul(ps[:], wk[c][:, bass.ts(e, 128)],
                                 x2t[c][:, bass.ts(tb * 2 + j, CH)],
                                 start=(c == 0), stop=(c == CT - 1))
            nc.vector.tensor_scalar_add(
                kt[e][:, bass.ts(tb * 2 + j, CH)], ps[:], bkr[:, e:e + 1])
        return run

    # head start on the first V tiles so pair 0's AV never waits
    for t in range(4):
        v_job(t)()
    pair_jobs = {
        0: [v_job(t) for t in range(4, KT)]
           + [k_half_job(1, tb, j) for tb in range(NKC) for j in range(2)],
        1: [k_half_job(2, tb, j) for tb in range(NKC) for j in range(2)],
        2: [k_half_job(3, tb, j) for tb in range(NKC) for j in range(2)],
    }

    # optional debug dumps of intermediates
    if "dbg" in io:
        for e in range(CT):
            nc.sync.dma_start(io["dbg_qt"][bass.ts(e, 128), :], qt[e][:])

    if True:
        yts = [py.tile([128, QC], F16, tag="yt", name=f"yt{e}")
               for e in range(CT)]

        def out_job(qi):
            # out[q, :] = sum_e Y^T[e, q] * WuT[e, :] + bu for one
            # 128-row q chunk.  pso rides the "s" tag ring.
            def run():
                pso = ps_s.tile([128, EMB], F32, tag="s", name="pso")
                for e in range(CT):
                    nc.tensor.matmul(pso[:], yts[e][:, bass.ts(qi, 128)],
                                     wu[e][:],
                                     start=(e == 0), stop=(e == CT - 1))
                osb = po.tile([128, EMB], F32, tag="o", name="osb")
                nc.vector.tensor_add(osb[:], pso[:], bub[:])
                nc.sync.dma_start(io["out"][bass.ts(qi, 128), :], osb[:])
            return run
        for pr in range(CT):  # head pair
            jobs = pair_jobs.pop(pr, [])
            for cb in range(NB):  # query-block pass: av shrinks to
                av = [ps_av.tile([65, QW], F32, tag="av", name="av")
                      for _ in range(2)]   # 2 banks, so "s" rings 3-deep

                def av_mms(kk, pt):
                    for hh in range(2):
                        nc.tensor.matmul(
                            av[hh][:],
                            v[:, kk, 2 * pr + hh, 0:65],
                            pt[:, bass.ts(hh, QW)],
                            start=(kk == 0), stop=(kk == KT - 1))

                pending = []  # AV issue lags ~3 steps so the av-buffer
                for kk in range(KT):   # WAR never head-blocks the PE
                    mkt = pm.tile([128, QW], F16, tag="mk", name="mk")
                    nc.sync.dma_start(mkt[:], io["maskT"][bass.ts(kk, 128),
                                                          bass.ts(cb, QW)])
                    ps = ps_s.tile([128, 2 * QW], F32, tag="s", name="ps_s")
                    for hh in range(2):  # head within pair
                        nc.tensor.matmul(
                            ps[:, bass.ts(hh, QW)],
                            kt[pr][bass.ds(64 * hh, 64), bass.ts(kk, 128)],
                            qt[pr][bass.ds(64 * hh, 64), bass.ts(cb, QW)],
                            start=True, stop=True,
                            tile_position=(64 * hh, 0))
                    if len(pending) >= 3:
                        av_mms(*pending.pop(0))
                    # deferred projection jobs: pair 0's first pass eats
                    # one per step (it is PE-bound anyway and the V jobs
                    # have early deadlines); elsewhere every other step.
                    # The 3-deep "s" ring absorbs the extra allocation
                    # without breaking the scores double-buffering.
                    if jobs and ((pr == 0 and cb == 0) or kk % 2 == 1):
                        jobs.pop(0)()

                    e16 = pe.tile([128, 2 * QW], F16, tag="E", name="e16")
                    nc.scalar.activation(e16[:], ps[:], EXP, scale=scale)
                    pt = ppp.tile([128, 2 * QW], F16, tag="P", name="pt")
                    nc.vector.tensor_mul(
                        pt[:].rearrange("p (h q) -> p h q", h=2),
                        e16[:].rearrange("p (h q) -> p h q", h=2),
                        mkt[:].unsqueeze(1).broadcast_to([128, 2, QW]))
                    pending.append((kk, pt))
                for p in pending:  # AV flush first: normalize waits on it
                    av_mms(*p)

                # normalize this pass: Y^T_h / r_h, r = ones row 64.
                # ysb copy runs on ACT (idle in the drain anyway) so DVE
                # keeps feeding the next pass's mask-mul stream.
                for hh in range(2):
                    ysb = pys.tile([65, QW], F32, tag="ys", name="ysb")
                    nc.scalar.copy(ysb[:], av[hh][:])
                    r0 = prr.tile([1, QW], F32, tag="r0", name="r0")
                    nc.vector.tensor_copy(r0[:], ysb[64:65, :])
                    rr32 = prr.tile([1, QW], F32, tag="rr32", name="rr32")
                    nc.vector.reciprocal_approx_fast(rr32[:], r0[:])
                    rr = prr.tile([1, QW], F16, tag="rr", name="rr")
                    with nc.allow_low_precision(reason="fp16 recip copy ok"):
                        nc.vector.tensor_copy(rr[:], rr32[:])
                    bc = ps_av.tile([64, QW], F32, tag="av", name="bc")
                    nc.tensor.matmul(bc[:], ones[:], rr[:],
                                     start=True, stop=True)
                    nc.vector.tensor_mul(
                        yts[pr][bass.ds(64 * hh, 64), bass.ts(cb, QW)],
                        ysb[0:64, :], bc[:])
            for job in jobs:      # any leftovers drain in the boundary
                job()
        # out-proj; the "s" ring is 3-deep so these pipeline fine
        for qi in range(QC // 128):
            out_job(qi)()


def build(cfg, num_devices=8, dbg=False):
    T, QC = cfg["T"], cfg["QC"]
    nc = bacc.Bacc("TRN2", target_bir_lowering=False, debug=False,
                   num_devices=num_devices)
    io = {
        "x1T": nc.dram_tensor("x1T", [EMB, QC], F16, kind="ExternalInput").ap(),
        "x2T": nc.dram_tensor("x2T", [EMB, T], F16, kind="ExternalInput").ap(),
        "maskT": nc.dram_tensor("maskT", [T, QC], F16, kind="ExternalInput").ap(),
        "wqT": nc.dram_tensor("wqT", [EMB, EMB], F16, kind="ExternalInput").ap(),
        "wkT": nc.dram_tensor("wkT", [EMB, EMB], F16, kind="ExternalInput").ap(),
        "wvT": nc.dram_tensor("wvT", [EMB, EMB], F16, kind="ExternalInput").ap(),
        "wuT": nc.dram_tensor("wuT", [EMB, EMB], F16, kind="ExternalInput").ap(),
        "bqr": nc.dram_tensor("bqr", [128, CT], F32, kind="ExternalInput").ap(),
        "bkr": nc.dram_tensor("bkr", [128, CT], F32, kind="ExternalInput").ap(),
        "bvb": nc.dram_tensor("bvb", [128, EMB], F32, kind="ExternalInput").ap(),
        "bub": nc.dram_tensor("bub", [128, EMB], F32, kind="ExternalInput").ap(),
        "out": nc.dram_tensor("out", [QC, EMB], F32, kind="ExternalOutput").ap(),
    }
    if dbg:
        io["dbg"] = True
        CH = min(512, QC)
        io["dbg_qt"] = nc.dram_tensor("dbg_qt", [EMB, QC], F16, kind="ExternalOutput").ap()
        io["dbg_kt"] = nc.dram_tensor("dbg_kt", [EMB, T], F16, kind="ExternalOutput").ap()
        io["dbg_v"] = nc.dram_tensor("dbg_v", [128, (T // 128) * H * 66], F16, kind="ExternalOutput").ap()
        io["dbg_e"] = nc.dram_tensor("dbg_e", [128, 2 * CH], F16, kind="ExternalOutput").ap()
        io["dbg_p"] = nc.dram_tensor("dbg_p", [128, 2 * CH], F16, kind="ExternalOutput").ap()
        io["dbg_y"] = nc.dram_tensor("dbg_y", [65, CH], F32, kind="ExternalOutput").ap()
    with tile.TileContext(nc) as tc:
        with ExitStack() as ctx:
            attention_body(ctx, tc, io, cfg)
    nc.compile()
    return nc


def host_prep(x1, x2, mask, Wq, bq, Wk, bk, Wv, bv, Wu, bu, cfg):
    """Build the 8 per-core input maps from full inputs."""
    T, QC = cfg["T"], cfg["QC"]
    shared = {
        "wqT": np.ascontiguousarray(Wq.T).astype(np.float16),
        "wkT": np.ascontiguousarray(Wk.T).astype(np.float16),
        "wvT": np.ascontiguousarray(Wv.T).astype(np.float16),
        "wuT": np.ascontiguousarray(Wu.T).astype(np.float16),
        "bqr": np.ascontiguousarray(bq.reshape(CT, 128).T).astype(np.float32),
        "bkr": np.ascontiguousarray(bk.reshape(CT, 128).T).astype(np.float32),
        "bvb": np.ascontiguousarray(np.broadcast_to(bv, (128, EMB))).astype(np.float32),
        "bub": np.ascontiguousarray(np.broadcast_to(bu, (128, EMB))).astype(np.float32),
    }
    x2T = [x2[b].T.astype(np.float16) for b in range(x1.shape[0])]
    in_maps = []
    n_cores = (x1.shape[0] * x1.shape[1]) // QC
    per_b = x1.shape[1] // QC
    for c in range(n_cores):
        b, q0 = c // per_b, (c % per_b) * QC
        in_maps.append(dict(
            shared,
            x1T=x1[b, q0:q0 + QC, :].T.astype(np.float16),
            x2T=x2T[b],
            maskT=mask[b, q0:q0 + QC, :].T.astype(np.float16),
        ))
    return in_maps


_NC_CACHE = {}


def kernel(x1, x2, mask, Wq, bq, Wk, bk, Wv, bv, Wu, bu):
    cfg = FULL_CFG
    B, TQ, _ = x1.shape
    in_maps = host_prep(np.asarray(x1, np.float32), np.asarray(x2, np.float32),
                        np.asarray(mask), np.asarray(Wq, np.float32),
                        np.asarray(bq, np.float32), np.asarray(Wk, np.float32),
                        np.asarray(bk, np.float32), np.asarray(Wv, np.float32),
                        np.asarray(bv, np.float32), np.asarray(Wu, np.float32),
                        np.asarray(bu, np.float32), cfg)
    key = (cfg["T"], cfg["QC"])
    if key not in _NC_CACHE:
        _NC_CACHE[key] = build(cfg)
    nc = _NC_CACHE[key]
    res = run_bass_kernel_spmd(nc, in_maps, core_ids=list(range(8)),
                               trace=bool(os.environ.get("KERNEL_TRACE")))
    if os.environ.get("KERNEL_TRACE"):
        kernel.last_exec_ns = res.exec_time_ns
        kernel.last_results = res
    out = np.empty((B, TQ, EMB), np.float32)
    per_b = TQ // cfg["QC"]
    for c in range(8):
        b, q0 = c // per_b, (c % per_b) * cfg["QC"]
        out[b, q0:q0 + cfg["QC"], :] = res.results[c]["out"]
    return out



# revision 44
# speedup vs baseline: 1.0070x; 1.0070x over previous
"""Multi-head cross attention on 8 trn2 NeuronCores.

Problem: B=2, T=4096, EMB=512, H=8 heads (head dim 64), fp32 I/O.
  q = x1 @ Wq.T + bq ; k,v from x2 ; S = q k^T / sqrt(512) ;
  softmax over keys with -1e10 masking ; out = (A v) @ Wu.T + bu.

Sharding: core c handles batch b = c//4 and query rows
[1024*(c%4), 1024*(c%4+1)).  Each core computes K,V for its batch in
full (4-way duplication), its own Q chunk, attention, and out-proj.

Device-side layout choices:
  - All matmul operands fp16 (PE rate is dtype-independent; fp16 halves
    DMA/SBUF and keeps ~1e-3 accuracy), accumulation fp32 in PSUM.
  - Scores computed TRANSPOSED, S^T[key, query]: contraction over the
    head dim requires Q^T/K^T (head-dim on partitions), which fall out
    of computing the projections transposed from x^T inputs (host
    pre-transposes x1/x2/W).  With keys on partitions, P^T = exp(S^T)*M^T
    feeds the AV matmul directly as its stationary-side contraction
    without any on-chip transposes.
  - Scores are small (|S| < ~1) so exp needs no max-subtraction; the
    1/sqrt(512) scale is folded into the ACT exp instruction.
  - V is stored interleaved [key, head, 65] with a ones column so the
    AV matmul also produces the softmax denominators r[q] (row 64).
  - Normalization is deferred: Y^T_h / r_h via reciprocal + a K=1
    broadcast matmul + one DVE multiply per (head, chunk).
  - 2 heads are packed per scores pass via tile_position row-tiling
    (contraction=64 -> rows 0-63 / 64-127 run concurrently).

Schedule (the exp stream on the scalar/ACT engine is the serial floor,
~1.04us per [128,1024] tile, ~265us total -- everything else hides
under it):
  - (key tile, query block) steps with the scores PSUM double-buffered
    (2 banks x 2 bufs) so exp never waits on the next scores matmul;
    AV matmuls issue 3 steps late so the av-buffer WAR at pair starts
    never head-blocks the PE queue.
  - Projections overlap attention: Q + K[e=0] run up front (DMA issue
    order tuned so the first matmul starts ~9us in), V tiles and
    K[e=1..3] are deferred jobs consumed one per odd step (pair 0
    carries V + K[1]; pairs 1/2 carry K[2]/K[3]) riding PE slack under
    the ACT pace; job PSUM shares the scores tag ring.
  - Out-projection alternates two PSUM tags for a 4-deep ring.
"""
import math
import os
from contextlib import ExitStack

import numpy as np

import concourse.bass as bass
import concourse.bacc as bacc
import concourse.tile as tile
import concourse.mybir as mybir
from concourse.bass_utils import run_bass_kernel_spmd

F16 = mybir.dt.float16
F32 = mybir.dt.float32
EXP = mybir.ActivationFunctionType.Exp

EMB, H, D, CT = 512, 8, 64, 4  # emb, heads, head dim, emb/128

FULL_CFG = dict(T=4096, QC=1024)  # keys per batch, query rows per core
MINI_CFG = dict(T=512, QC=256)


def attention_body(ctx, tc, io, cfg):
    nc = tc.nc
    T, QC = cfg["T"], cfg["QC"]
    KT = T // 128            # key tiles
    NG = KT // 2             # key-tile groups of 2
    CH = min(512, QC)        # query chunk width
    NCH = QC // CH
    scale = 1.0 / math.sqrt(EMB)

    pw = ctx.enter_context(tc.tile_pool(name="w", bufs=1))
    pk = ctx.enter_context(tc.tile_pool(name="kt", bufs=1))
    pv = ctx.enter_context(tc.tile_pool(name="v", bufs=1))
    pq = ctx.enter_context(tc.tile_pool(name="qt", bufs=1))

    # persistent weights / biases / constants
    wq = [pw.tile([128, EMB], F16, tag=f"wq{i}", name=f"wq{i}") for i in range(CT)]
    wk = [pw.tile([128, EMB], F16, tag=f"wk{i}", name=f"wk{i}") for i in range(CT)]
    wv = [pw.tile([128, EMB], F16, tag=f"wv{i}", name=f"wv{i}") for i in range(CT)]
    wu = [pw.tile([128, EMB], F16, tag=f"wu{i}", name=f"wu{i}") for i in range(CT)]
    # DMA priority order: what phase A needs first (wq+x1 for Q, then
    # wk+x2 for K e=0); V/out weights follow, mask prefetch queues after.
    bqr = pw.tile([128, CT], F32, tag="bqr", name="bqr")
    bkr = pw.tile([128, CT], F32, tag="bkr", name="bkr")
    bvb = pw.tile([128, EMB], F32, tag="bvb", name="bvb")
    bub = pw.tile([128, EMB], F32, tag="bub", name="bub")
    ones = pw.tile([1, D], F16, tag="ones", name="ones")
    nc.vector.memset(ones[:], 1.0)

    # persistent K^T [emb, T], V [T, head, 65(+pad)], Q^T [emb, QC]
    kt = [pk.tile([128, T], F16, tag=f"kt{i}", name=f"kt{i}") for i in range(CT)]
    v = pv.tile([128, KT, H, 66], F16, tag="v", name="v")
    nc.vector.memset(v[:, :, :, 64:65], 1.0)
    qt = [pq.tile([128, QC], F16, tag=f"qt{i}", name=f"qt{i}") for i in range(CT)]

    px = ctx.enter_context(tc.tile_pool(name="x", bufs=1))
    x2t = [px.tile([128, T], F16, tag=f"x2t{i}", name=f"x2t{i}") for i in range(CT)]
    x1t = [px.tile([128, QC], F16, tag=f"x1t{i}", name=f"x1t{i}") for i in range(CT)]
    # DMA issue order is the critical path to the first matmul: each
    # dma_start costs ~600ns of sync-engine descriptor gen, so x1/wq
    # interleave first (Q proj c=0 starts ~9us in), then wk + x2
    # quarter 1 (K e=0 tb=0), then the rest.
    for i in range(CT):
        nc.sync.dma_start(x1t[i][:], io["x1T"][bass.ts(i, 128), :])
        nc.sync.dma_start(wq[i][:], io["wqT"][bass.ts(i, 128), :])
    nc.sync.dma_start(bqr[:], io["bqr"][:, :])
    for i in range(CT):
        nc.sync.dma_start(wk[i][:], io["wkT"][bass.ts(i, 128), :])
    nc.sync.dma_start(bkr[:], io["bkr"][:, :])
    for hf in range(4):   # quarters: K chunk tb only needs its quarter
        for i in range(CT):
            nc.sync.dma_start(x2t[i][:, bass.ts(hf, T // 4)],
                              io["x2T"][bass.ts(i, 128), bass.ts(hf, T // 4)])

    # attention layout: (key tile, query block) steps.  Scores PSUM is
    # double-buffered at [128, 2*QW] (2 banks x 2 bufs) so the exp ACT
    # stream (the hard serial floor) runs back-to-back.  Projections are
    # folded into the same "s" PSUM tag: Q and K[e=0] run up front, the
    # V tiles interleave into pair 0's steps one tile per step, and
    # K[e=pr+1] runs in the pair boundary where ACT drains anyway.
    QW = min(512, QC)        # matmul moving width (PSUM bank limit)
    NB = QC // QW            # query blocks
    ps_s = ctx.enter_context(tc.tile_pool(name="ps_s", bufs=3, space="PSUM"))
    ps_av = ctx.enter_context(tc.tile_pool(name="ps_av", bufs=2, space="PSUM"))
    pe = ctx.enter_context(tc.tile_pool(name="pe", bufs=3))
    ppp = ctx.enter_context(tc.tile_pool(name="ppp", bufs=6))
    pm = ctx.enter_context(tc.tile_pool(name="pm", bufs=min(KT, 6)))
    py = ctx.enter_context(tc.tile_pool(name="py", bufs=CT))
    pys = ctx.enter_context(tc.tile_pool(name="pys", bufs=2))
    prr = ctx.enter_context(tc.tile_pool(name="prr", bufs=2))
    po = ctx.enter_context(tc.tile_pool(name="po", bufs=3))

    # ---- phase A: Q^T (all e) and K^T e=0 ----
    for e in range(CT):
        ps = ps_s.tile([128, 2 * CH], F32, tag="s", name="psq")
        for c in range(CT):
            for t in range(QC // CH):
                nc.tensor.matmul(ps[:, bass.ts(t, CH)],
                                 wq[c][:, bass.ts(e, 128)],
                                 x1t[c][:, bass.ts(t, CH)],
                                 start=(c == 0), stop=(c == CT - 1))
        nc.vector.tensor_scalar_add(qt[e][:], ps[:, 0:QC], bqr[:, e:e + 1])
    NKC = T // CH // 2       # K chunk pairs per e-tile
    for tb in range(NKC):
        ps = ps_s.tile([128, 2 * CH], F32, tag="s", name="psk")
        for c in range(CT):
            for j in range(2):
                nc.tensor.matmul(ps[:, bass.ts(j, CH)], wk[c][:, 0:128],
                                 x2t[c][:, bass.ts(tb * 2 + j, CH)],
                                 start=(c == 0), stop=(c == CT - 1))
        nc.vector.tensor_scalar_add(kt[0][:, bass.ds(tb * 2 * CH, 2 * CH)],
                                    ps[:], bkr[:, 0:1])

    # remaining weights / biases can load after phase A is under way
    for i in range(CT):
        nc.sync.dma_start(wv[i][:], io["wvT"][bass.ts(i, 128), :])
    nc.sync.dma_start(bvb[:], io["bvb"][:, :])
    for i in range(CT):
        nc.sync.dma_start(wu[i][:], io["wuT"][bass.ts(i, 128), :])
    nc.sync.dma_start(bub[:], io["bub"][:, :])

    # ---- deferred projection jobs (ride odd-step PE slack) ----
    def v_job(t):
        def run():
            ps = ps_s.tile([128, EMB], F32, tag="s", name="psv")
            for c in range(CT):
                nc.tensor.matmul(ps[:], x2t[c][:, bass.ts(t, 128)], wv[c][:],
                                 start=(c == 0), stop=(c == CT - 1))
            nc.vector.tensor_add(
                v[:, t, :, 0:64],
                ps[:].rearrange("p (h d) -> p h d", h=H),
                bvb[:].rearrange("p (h d) -> p h d", h=H))
        return run

    def k_half_job(e, tb, j):
        def run():
            ps = ps_s.tile([128, CH], F32, tag="s", name="psk")
            for c in range(CT):
                nc.tensor.matmul(ps[:], wk[c][:, bass.ts(e, 128)],
                                 x2t[c][:, bass.ts(tb * 2 + j, CH)],
                                 start=(c == 0), stop=(c == CT - 1))
            nc.vector.tensor_scalar_add(
                kt[e][:, bass.ts(tb * 2 + j, CH)], ps[:], bkr[:, e:e + 1])
        return run

    # head start on the first V tiles so pair 0's AV never waits
    for t in range(2):
        v_job(t)()
    pair_jobs = {
        0: [v_job(t) for t in range(2, KT)]
           + [k_half_job(1, tb, j) for tb in range(NKC) for j in range(2)],
        1: [k_half_job(2, tb, j) for tb in range(NKC) for j in range(2)],
        2: [k_half_job(3, tb, j) for tb in range(NKC) for j in range(2)],
    }

    # optional debug dumps of intermediates
    if "dbg" in io:
        for e in range(CT):
            nc.sync.dma_start(io["dbg_qt"][bass.ts(e, 128), :], qt[e][:])

    if True:
        yts = [py.tile([128, QC], F16, tag="yt", name=f"yt{e}")
               for e in range(CT)]

        def out_job(qi):
            # out[q, :] = sum_e Y^T[e, q] * WuT[e, :] + bu for one
            # 128-row q chunk.  pso rides the "s" tag ring.
            def run():
                pso = ps_s.tile([128, EMB], F32, tag="s", name="pso")
                for e in range(CT):
                    nc.tensor.matmul(pso[:], yts[e][:, bass.ts(qi, 128)],
                                     wu[e][:],
                                     start=(e == 0), stop=(e == CT - 1))
                osb = po.tile([128, EMB], F32, tag="o", name="osb")
                nc.vector.tensor_add(osb[:], pso[:], bub[:])
                nc.sync.dma_start(io["out"][bass.ts(qi, 128), :], osb[:])
            return run
        for pr in range(CT):  # head pair
            jobs = pair_jobs.pop(pr, [])
            for cb in range(NB):  # query-block pass: av shrinks to
                av = [ps_av.tile([65, QW], F32, tag="av", name="av")
                      for _ in range(2)]   # 2 banks, so "s" rings 3-deep

                def av_mms(kk, pt):
                    for hh in range(2):
                        nc.tensor.matmul(
                            av[hh][:],
                            v[:, kk, 2 * pr + hh, 0:65],
                            pt[:, bass.ts(hh, QW)],
                            start=(kk == 0), stop=(kk == KT - 1))

                pending = []  # AV issue lags ~3 steps so the av-buffer
                for kk in range(KT):   # WAR never head-blocks the PE
                    mkt = pm.tile([128, QW], F16, tag="mk", name="mk")
                    nc.sync.dma_start(mkt[:], io["maskT"][bass.ts(kk, 128),
                                                          bass.ts(cb, QW)])
                    ps = ps_s.tile([128, 2 * QW], F32, tag="s", name="ps_s")
                    for hh in range(2):  # head within pair
                        nc.tensor.matmul(
                            ps[:, bass.ts(hh, QW)],
                            kt[pr][bass.ds(64 * hh, 64), bass.ts(kk, 128)],
                            qt[pr][bass.ds(64 * hh, 64), bass.ts(cb, QW)],
                            start=True, stop=True,
                            tile_position=(64 * hh, 0))
                    if len(pending) >= 4:
                        av_mms(*pending.pop(0))
                    # deferred projection jobs: pair 0's first pass eats
                    # one per step (it is PE-bound anyway and the V jobs
                    # have early deadlines); elsewhere every other step.
                    # The 3-deep "s" ring absorbs the extra allocation
                    # without breaking the scores double-buffering.
                    if jobs and ((pr == 0 and cb == 0) or kk % 2 == 1):
                        jobs.pop(0)()

                    e16 = pe.tile([128, 2 * QW], F16, tag="E", name="e16")
                    nc.scalar.activation(e16[:], ps[:], EXP, scale=scale)
                    pt = ppp.tile([128, 2 * QW], F16, tag="P", name="pt")
                    nc.vector.tensor_mul(
                        pt[:].rearrange("p (h q) -> p h q", h=2),
                        e16[:].rearrange("p (h q) -> p h q", h=2),
                        mkt[:].unsqueeze(1).broadcast_to([128, 2, QW]))
                    pending.append((kk, pt))
                for p in pending:  # AV flush first: normalize waits on it
                    av_mms(*p)

                # normalize this pass: Y^T_h / r_h, r = ones row 64.
                # ysb copy runs on ACT (idle in the drain anyway) so DVE
                # keeps feeding the next pass's mask-mul stream.
                for hh in range(2):
                    ysb = pys.tile([65, QW], F32, tag="ys", name="ysb")
                    nc.scalar.copy(ysb[:], av[hh][:])
                    r0 = prr.tile([1, QW], F32, tag="r0", name="r0")
                    nc.vector.tensor_copy(r0[:], ysb[64:65, :])
                    rr32 = prr.tile([1, QW], F32, tag="rr32", name="rr32")
                    nc.vector.reciprocal_approx_fast(rr32[:], r0[:])
                    rr = prr.tile([1, QW], F16, tag="rr", name="rr")
                    with nc.allow_low_precision(reason="fp16 recip copy ok"):
                        nc.vector.tensor_copy(rr[:], rr32[:])
                    bc = ps_av.tile([64, QW], F32, tag="av", name="bc")
                    nc.tensor.matmul(bc[:], ones[:], rr[:],
                                     start=True, stop=True)
                    nc.vector.tensor_mul(
                        yts[pr][bass.ds(64 * hh, 64), bass.ts(cb, QW)],
                        ysb[0:64, :], bc[:])
            for job in jobs:      # any leftovers drain in the boundary
                job()
        # out-proj; the "s" ring is 3-deep so these pipeline fine
        for qi in range(QC // 128):
            out_job(qi)()


def build(cfg, num_devices=8, dbg=False):
    T, QC = cfg["T"], cfg["QC"]
    nc = bacc.Bacc("TRN2", target_bir_lowering=False, debug=False,
                   num_devices=num_devices)
    io = {
        "x1T": nc.dram_tensor("x1T", [EMB, QC], F16, kind="ExternalInput").ap(),
        "x2T": nc.dram_tensor("x2T", [EMB, T], F16, kind="ExternalInput").ap(),
        "maskT": nc.dram_tensor("maskT", [T, QC], F16, kind="ExternalInput").ap(),
        "wqT": nc.dram_tensor("wqT", [EMB, EMB], F16, kind="ExternalInput").ap(),
        "wkT": nc.dram_tensor("wkT", [EMB, EMB], F16, kind="ExternalInput").ap(),
        "wvT": nc.dram_tensor("wvT", [EMB, EMB], F16, kind="ExternalInput").ap(),
        "wuT": nc.dram_tensor("wuT", [EMB, EMB], F16, kind="ExternalInput").ap(),
        "bqr": nc.dram_tensor("bqr", [128, CT], F32, kind="ExternalInput").ap(),
        "bkr": nc.dram_tensor("bkr", [128, CT], F32, kind="ExternalInput").ap(),
        "bvb": nc.dram_tensor("bvb", [128, EMB], F32, kind="ExternalInput").ap(),
        "bub": nc.dram_tensor("bub", [128, EMB], F32, kind="ExternalInput").ap(),
        "out": nc.dram_tensor("out", [QC, EMB], F32, kind="ExternalOutput").ap(),
    }
    if dbg:
        io["dbg"] = True
        CH = min(512, QC)
        io["dbg_qt"] = nc.dram_tensor("dbg_qt", [EMB, QC], F16, kind="ExternalOutput").ap()
        io["dbg_kt"] = nc.dram_tensor("dbg_kt", [EMB, T], F16, kind="ExternalOutput").ap()
        io["dbg_v"] = nc.dram_tensor("dbg_v", [128, (T // 128) * H * 66], F16, kind="ExternalOutput").ap()
        io["dbg_e"] = nc.dram_tensor("dbg_e", [128, 2 * CH], F16, kind="ExternalOutput").ap()
        io["dbg_p"] = nc.dram_tensor("dbg_p", [128, 2 * CH], F16, kind="ExternalOutput").ap()
        io["dbg_y"] = nc.dram_tensor("dbg_y", [65, CH], F32, kind="ExternalOutput").ap()
    with tile.TileContext(nc) as tc:
        with ExitStack() as ctx:
            attention_body(ctx, tc, io, cfg)
    nc.compile()
    return nc


def host_prep(x1, x2, mask, Wq, bq, Wk, bk, Wv, bv, Wu, bu, cfg):
    """Build the 8 per-core input maps from full inputs."""
    T, QC = cfg["T"], cfg["QC"]
    shared = {
        "wqT": np.ascontiguousarray(Wq.T).astype(np.float16),
        "wkT": np.ascontiguousarray(Wk.T).astype(np.float16),
        "wvT": np.ascontiguousarray(Wv.T).astype(np.float16),
        "wuT": np.ascontiguousarray(Wu.T).astype(np.float16),
        "bqr": np.ascontiguousarray(bq.reshape(CT, 128).T).astype(np.float32),
        "bkr": np.ascontiguousarray(bk.reshape(CT, 128).T).astype(np.float32),
        "bvb": np.ascontiguousarray(np.broadcast_to(bv, (128, EMB))).astype(np.float32),
        "bub": np.ascontiguousarray(np.broadcast_to(bu, (128, EMB))).astype(np.float32),
    }
    x2T = [x2[b].T.astype(np.float16) for b in range(x1.shape[0])]
    in_maps = []
    n_cores = (x1.shape[0] * x1.shape[1]) // QC
    per_b = x1.shape[1] // QC
    for c in range(n_cores):
        b, q0 = c // per_b, (c % per_b) * QC
        in_maps.append(dict(
            shared,
            x1T=x1[b, q0:q0 + QC, :].T.astype(np.float16),
            x2T=x2T[b],
            maskT=mask[b, q0:q0 + QC, :].T.astype(np.float16),
        ))
    return in_maps


_NC_CACHE = {}


def kernel(x1, x2, mask, Wq, bq, Wk, bk, Wv, bv, Wu, bu):
    cfg = FULL_CFG
    B, TQ, _ = x1.shape
    in_maps = host_prep(np.asarray(x1, np.float32), np.asarray(x2, np.float32),
                        np.asarray(mask), np.asarray(Wq, np.float32),
                        np.asarray(bq, np.float32), np.asarray(Wk, np.float32),
                        np.asarray(bk, np.float32), np.asarray(Wv, np.float32),
                        np.asarray(bv, np.float32), np.asarray(Wu, np.float32),
                        np.asarray(bu, np.float32), cfg)
    key = (cfg["T"], cfg["QC"])
    if key not in _NC_CACHE:
        _NC_CACHE[key] = build(cfg)
    nc = _NC_CACHE[key]
    res = run_bass_kernel_spmd(nc, in_maps, core_ids=list(range(8)),
                               trace=bool(os.environ.get("KERNEL_TRACE")))
    if os.environ.get("KERNEL_TRACE"):
        kernel.last_exec_ns = res.exec_time_ns
        kernel.last_results = res
    out = np.empty((B, TQ, EMB), np.float32)
    per_b = TQ // cfg["QC"]
    for c in range(8):
        b, q0 = c // per_b, (c % per_b) * cfg["QC"]
        out[b, q0:q0 + cfg["QC"], :] = res.results[c]["out"]
    return out

